# revision 19
# baseline (speedup 1.0000x reference)
"""Trainium2 Bass kernel for nn_NodeAttDiff (segment-reduce node attention).

Math (reference):
    e1, e2 = out_gnn[:N], out_gnn[N:]          # N = 200000, D = 256
    diff   = e1 - e2
    h      = relu([e1 e2 diff] @ W1 + b1)      # folded: e1@WA + e2@WB, WA=W1a+W1c, WB=W1b-W1c
    raw    = (h @ W2 + b2)[:, 0]
    att    = segment_softmax(raw, batch)       # 512 contiguous segments (batch sorted)
    out    = segment_sum(att[:,None] * diff)   # [512, 256]

Device strategy (8 cores, graph-partitioned data parallel):
    - 64 graphs / core; each core gets its contiguous node slice (padded to a
      common capacity, pad nodes carry out-of-range segment id -> dropped).
    - Softmax max-subtraction is skipped (raw is O(5); exp is safe in fp32) and
      normalization is algebraic:  out_g = (sum_n w_n diff_n) / (sum_n w_n),
      w_n = exp(raw_n + b2)  -- so no per-node att materialization is needed.
    - Host ships THREE tensors per core (fp16, ~39 MB total vs 360 GB/s HBM):
        e1t/e2t feature-major (merged dram tensor e12) feed the z GEMM;
        dn node-major [cap/128,128,258] = [diff | 1 | 0] rows feeds the
        segment matmul directly (no PE transposes, no DVE sub/copies; the
        denominator "ones" column ships inside the same DMA).
    - Software pipeline, one group = 2x512-node tiles, all cross-engine
      dependencies get >= 1 full group of slack so the PE never stalls:
        iteration g:  PE:  raw(g-1) x4 -> seg(g-2) x8 -> z(g) x8
                      ACT: exp(g-1) x2, relu(g) x2 (PSUM pair -> fp16 SBUF)
                      DMA: e12(g+2), dn(g+2); XBAR-transpose ew(g-1) -> ewt
                      DVE: Sw(g-1) = (iota == seg_id) * ewt  x8
      The ew row pair [16,1024] is transposed by the DMA crossbar (14 ns/tile)
      instead of PE outer-product matmuls; PSUM = 3x z (2 banks each) +
      raw + seg = 8 banks.
    - Tail: out = seg[:,0:256] * recip(max(seg[:,256], eps)), DMA out [gw,256].
"""

import os
import numpy as np

NUM_GRAPHS = 512
N_CORES = 8
D = 256
TILE_N = 512  # nodes per tile
DN_W = D + 2  # diff row + [1, 0]


_CACHE = {}


def _build_program(cap: int, gw: int, use_b1: bool = False):
    """Build + compile the SPMD Bass program; `cap` nodes and a `gw`-graph
    window per core."""
    if (cap, gw, use_b1) in _CACHE:
        return _CACHE[(cap, gw, use_b1)]

    from contextlib import ExitStack
    import concourse.bass as bass
    import concourse.tile as tile
    import concourse.bacc as bacc
    import concourse.mybir as mybir

    f32 = mybir.dt.float32
    f16 = mybir.dt.float16
    AF = mybir.ActivationFunctionType
    ALU = mybir.AluOpType

    assert cap % (2 * TILE_N) == 0
    n_grp = cap // (2 * TILE_N)
    n_tiles = cap // TILE_N
    n_cols = cap // 128  # bm columns
    n_blk = cap // 128   # 128-node blocks

    nc = bacc.Bacc("TRN2", target_bir_lowering=False, debug=False,
                   num_devices=N_CORES)

    e12_d = nc.dram_tensor("e12", [2, 2, 128, cap], f16, kind="ExternalInput").ap()
    dn_d = nc.dram_tensor("dn", [n_blk, 128, DN_W], f16, kind="ExternalInput").ap()
    bm_d = nc.dram_tensor("bm", [128, n_cols], f32, kind="ExternalInput").ap()
    wa_d = nc.dram_tensor("wa", [2, 2, 128, 128], f16, kind="ExternalInput").ap()
    wb_d = nc.dram_tensor("wb", [2, 2, 128, 128], f16, kind="ExternalInput").ap()
    w2_d = nc.dram_tensor("w2", [2, 128, 2], f16, kind="ExternalInput").ap()
    b1_d = nc.dram_tensor("b1", [2, 128, 1], f32, kind="ExternalInput").ap()
    b2_d = nc.dram_tensor("b2", [1, 1], f32, kind="ExternalInput").ap()
    iota_d = nc.dram_tensor("iota", [128, gw], f16, kind="ExternalInput").ap()
    out_d = nc.dram_tensor("out", [gw, D], f32, kind="ExternalOutput").ap()

    with tile.TileContext(nc) as tc:
        with ExitStack() as ctx:
            consts = ctx.enter_context(tc.tile_pool(name="consts", bufs=1))
            epool = ctx.enter_context(tc.tile_pool(name="epool", bufs=6))
            dpool = ctx.enter_context(tc.tile_pool(name="dpool", bufs=7))
            hpool = ctx.enter_context(tc.tile_pool(name="hpool", bufs=6))
            spool = ctx.enter_context(tc.tile_pool(name="spool", bufs=6))
            zpool = ctx.enter_context(
                tc.tile_pool(name="zpool", bufs=3, space=bass.MemorySpace.PSUM))
            rawpool = ctx.enter_context(
                tc.tile_pool(name="rawpool", bufs=1, space=bass.MemorySpace.PSUM))
            segpool = ctx.enter_context(
                tc.tile_pool(name="segpool", bufs=1, space=bass.MemorySpace.PSUM))

            # ---- constants (wa/wb first: first z matmul needs them) ----
            wa = consts.tile([128, 2, 2, 128], f16, tag="wa")
            wb = consts.tile([128, 2, 2, 128], f16, tag="wb")
            w2 = consts.tile([128, 2, 2], f16, tag="w2")
            b1 = consts.tile([128, 2, 1], f32, tag="b1")
            b2 = consts.tile([1, 1], f32, tag="b2")
            iota = consts.tile([128, gw], f16, tag="iota")
            bm = consts.tile([128, n_cols], f32, tag="bm")
            nc.sync.dma_start(wa[:], wa_d.rearrange("k m p n -> p k m n"))
            nc.sync.dma_start(wb[:], wb_d.rearrange("k m p n -> p k m n"))

            # seg layout: cols 0:256 weighted diff sums, col 256 exp-sums
            seg = segpool.tile([gw, DN_W], f32, tag="seg")

            e_tiles = {}
            d_tiles = {}

            def issue_dma(g):
                if g >= n_grp:
                    return
                e12 = epool.tile([128, 2, 2, 2 * TILE_N], f16, tag="e12")
                dn = dpool.tile([128, 8, DN_W], f16, tag="dn")
                gsl = bass.ts(g, 2 * TILE_N)
                nc.sync.dma_start(
                    e12[:], e12_d[:, :, :, gsl].rearrange("s k p n -> p s k n"))
                nc.sync.dma_start(dn[:], dn_d[bass.ts(g, 8)].rearrange("b p f -> p b f"))
                e_tiles[g] = e12
                d_tiles[g] = dn

            issue_dma(0)
            # remaining consts after the first data group
            nc.sync.dma_start(w2[:], w2_d.rearrange("m p n -> p m n"))
            nc.sync.dma_start(b1[:], b1_d.rearrange("m p n -> p m n"))
            nc.sync.dma_start(b2[:], b2_d[:])
            nc.sync.dma_start(iota[:], iota_d[:])
            nc.sync.dma_start(bm[:], bm_d[:])
            issue_dma(1)

            h_tiles = {}
            ewt_tiles = {}
            sw_tiles = {}
            zc_tiles = {}
            seg_started = [False]

            # every cross-engine dependency gets >= 1 full iteration of slack:
            # raw(g) consumes h(g) two iterations after relu(g) was issued,
            # seg(g) consumes sw(g) one full iteration after the DVE batch
            # that produced it, so the in-order PE queue never stalls and the
            # tensor engine stays at its ramped p-state.
            for it in range(n_grp + 3):
                gz = it          # z / relu phase
                gr = it - 2      # raw / exp / xbar / sw phase
                gs = it - 3      # seg accumulate phase

                # ---- prefetch group g+2 (ahead of this iteration's XBAR so
                # the in-order sync queue keeps the DMA runahead at 2 groups)
                issue_dma(it + 2)

                # ---- raw(g-1): 4 matmuls [2,512] (h has a full group of slack)
                if 0 <= gr < n_grp:
                    rawp = rawpool.tile([64, TILE_N], f32, tag="raw")
                    for ti in range(2):
                        h = h_tiles[(gr, ti)]
                        nc.tensor.matmul(rawp[32 * ti:32 * ti + 2, :],
                                         w2[:, 0, :], h[:, 0, :],
                                         start=True, stop=False,
                                         skip_group_check=True)
                        nc.tensor.matmul(rawp[32 * ti:32 * ti + 2, :],
                                         w2[:, 1, :], h[:, 1, :],
                                         start=False, stop=True,
                                         skip_group_check=True)
                    # ew = exp(raw + b2) -> rows 0 of a [16, 1024] strip
                    ew = spool.tile([16, 2, TILE_N], f16, tag="ew")
                    for ti in range(2):
                        nc.scalar.activation(ew[0:1, ti, :],
                                             rawp[32 * ti:32 * ti + 1, :],
                                             AF.Exp, bias=b2[:], scale=1.0)
                    # ewt[p, blk, 0] = ew[0, blk*128 + p]  (DMA crossbar)
                    ewt = spool.tile([128, 8, 16], f16, tag="ewt")
                    nc.sync.dma_start_transpose(ewt[:], ew[:])
                    # fp32 copy of the 8 useful columns (tensor_scalar scalar2
                    # must be fp32 for is_equal)
                    ewt32 = spool.tile([128, 8, 1], f32, tag="ewt32")
                    nc.vector.tensor_copy(ewt32[:], ewt[:, :, 0:1])
                    ewt_tiles[gr] = ewt32

                # ---- seg(g-2): 8 matmuls into the whole-core accumulator
                if 0 <= gs < n_grp:
                    dnt = d_tiles.pop(gs)
                    for ti in range(2):
                        sw = sw_tiles.pop((gs, ti))
                        for b in range(4):
                            nc.tensor.matmul(seg[:], sw[:, b, :],
                                             dnt[:, 4 * ti + b, :],
                                             start=not seg_started[0],
                                             stop=(gs == n_grp - 1 and ti == 1
                                                   and b == 3),
                                             skip_group_check=True)
                            seg_started[0] = True

                # ---- z(g): 8 matmuls, each weight chunk streamed to both tiles
                if gz < n_grp:
                    e12 = e_tiles.pop(gz)
                    zc = [zpool.tile([128, 2, TILE_N], f32, tag="zr",
                                     name=f"z_{gz}_{ti}") for ti in range(2)]
                    for m in range(2):
                        for wi in range(4):
                            wmat = wa if wi < 2 else wb
                            s = wi // 2  # e1 or e2
                            k = wi % 2
                            for ti in range(2):
                                nc.tensor.matmul(
                                    zc[ti][:, m, :], wmat[:, k, m, :],
                                    e12[:, s, k, bass.ts(ti, TILE_N)],
                                    start=(wi == 0), stop=(wi == 3))
                    # h = relu(z + b1); fast path (b1 == 0) fuses both banks
                    for ti in range(2):
                        h = hpool.tile([128, 2, TILE_N], f16, tag="h")
                        if use_b1:
                            for m in range(2):
                                nc.scalar.activation(h[:, m, :], zc[ti][:, m, :],
                                                     AF.Relu, bias=b1[:, m, :],
                                                     scale=1.0)
                        else:
                            nc.scalar.activation(h[:], zc[ti][:], AF.Relu,
                                                 scale=1.0)
                        h_tiles[(gz, ti)] = h
                    zc_tiles[gz] = zc

                # ---- Sw(g-1) on DVE (after the XBAR above)
                if 0 <= gr < n_grp:
                    ewt = ewt_tiles.pop(gr)
                    for ti in range(2):
                        t = 2 * gr + ti
                        sw = spool.tile([128, 4, gw], f16, tag="sw")
                        for b in range(4):
                            nc.vector.tensor_scalar(
                                sw[:, b, :], iota[:],
                                bm[:, 4 * t + b:4 * t + b + 1],
                                ewt[:, 4 * ti + b, :],
                                op0=ALU.is_equal, op1=ALU.mult)
                        sw_tiles[(gr, ti)] = sw

            # tail: out = seg[:, 0:256] / max(seg[:, 256], eps)
            ssum = spool.tile([gw, 1], f32, tag="ssum")
            nc.vector.tensor_scalar_max(ssum[:], seg[:, D:D + 1], 1e-30)
            rec = spool.tile([gw, 1], f32, tag="rec")
            nc.vector.reciprocal(rec[:], ssum[:])
            ot = spool.tile([gw, D], f32, tag="ot")
            nc.vector.tensor_scalar_mul(ot[:], seg[:, 0:D], rec[:])
            nc.sync.dma_start(out_d[:], ot[:])

    nc.compile()
    _CACHE[(cap, gw, use_b1)] = nc
    return nc


def _prepare(out_gnn, batch_input, W1, b1, W2, b2):
    out_gnn = np.asarray(out_gnn, dtype=np.float32)
    batch = np.asarray(batch_input, dtype=np.int64)
    W1 = np.asarray(W1, dtype=np.float32)
    b1 = np.asarray(b1, dtype=np.float32)
    W2 = np.asarray(W2, dtype=np.float32)
    b2 = np.asarray(b2, dtype=np.float32)
    use_b1 = bool(b1.any())

    half = out_gnn.shape[0] // 2
    batch = batch[:half]
    e1_all, e2_all = out_gnn[:half], out_gnn[half:]

    # Node-balanced, graph-aligned contiguous cuts. Core c handles graphs
    # [gcut[c], gcut[c+1]) and the matching contiguous node range. The
    # sorted batch may populate only a prefix of the 512 graphs, so cuts
    # are chosen by node mass, not by fixed graph ranges.
    counts = np.bincount(batch, minlength=NUM_GRAPHS)
    ccum = np.concatenate([[0], np.cumsum(counts)])  # node offset per graph
    # only graphs up to the last populated one get device windows; trailing
    # empty graphs stay host-side zeros
    g_used = int(np.max(np.nonzero(counts)[0])) + 1 if counts.any() else 1
    gcut = np.zeros(N_CORES + 1, dtype=np.int64)
    gcut[N_CORES] = g_used
    for c in range(1, N_CORES):
        g = int(np.searchsorted(ccum, ccum[g_used] * c / N_CORES, side="left"))
        gcut[c] = min(max(g, gcut[c - 1]), g_used)
    spans = gcut[1:] - gcut[:-1]
    if spans.max() > 128:
        # node-balanced cuts gave an oversized graph window (pathological
        # distribution) -- fall back to an even graph split of [0, g_used)
        gcut = np.round(np.linspace(0, g_used, N_CORES + 1)).astype(np.int64)
        spans = gcut[1:] - gcut[:-1]
        if spans.max() > 128:
            raise ValueError(f"graph window {spans.max()} > 128 unsupported")

    nbounds = ccum[gcut]  # node boundaries per core
    gw = int(max(2, ((spans.max() + 1) // 2) * 2))
    max_n = int((nbounds[1:] - nbounds[:-1]).max())
    grp = 2 * TILE_N
    cap = max(grp, ((max_n + grp - 1) // grp) * grp)

    nc = _build_program(cap, gw, use_b1)

    # host-folded MLP weights (fp64 for exactness, then fp16)
    W1a = W1[0:D].astype(np.float64)
    W1b = W1[D:2 * D].astype(np.float64)
    W1c = W1[2 * D:3 * D].astype(np.float64)
    WA = (W1a + W1c).astype(np.float32)
    WB = (W1b - W1c).astype(np.float32)

    def chunk4(w):  # [256,256] -> [ki, mo, 128, 128]
        return np.ascontiguousarray(
            w.astype(np.float16).reshape(2, 128, 2, 128).transpose(0, 2, 1, 3))

    common = {
        "wa": chunk4(WA),
        "wb": chunk4(WB),
        "w2": np.ascontiguousarray(np.concatenate(
            [W2.astype(np.float16).reshape(2, 128, 1),
             np.zeros((2, 128, 1), np.float16)], axis=2)),
        "b1": np.ascontiguousarray(b1.reshape(2, 128, 1)),
        "b2": b2.reshape(1, 1).astype(np.float32),
        "iota": np.broadcast_to(np.arange(gw, dtype=np.float16), (128, gw)).copy(),
    }

    in_maps = []
    for c in range(N_CORES):
        s, e = int(nbounds[c]), int(nbounds[c + 1])
        n_c = e - s
        e12 = np.zeros((2, 2, 128, cap), dtype=np.float16)
        e12[0, :, :, :n_c] = e1_all[s:e].astype(np.float16).T.reshape(2, 128, n_c)
        e12[1, :, :, :n_c] = e2_all[s:e].astype(np.float16).T.reshape(2, 128, n_c)
        dn = np.zeros((cap, DN_W), dtype=np.float16)
        dn[:n_c, :D] = (e1_all[s:e] - e2_all[s:e]).astype(np.float16)
        dn[:, D] = 1.0  # denominator ones column (pad rows get Sw == 0 anyway)
        bmv = np.full(cap, 999.0, dtype=np.float32)
        bmv[:n_c] = (batch[s:e] - gcut[c]).astype(np.float32)
        in_maps.append({
            "e12": e12,
            "dn": dn.reshape(cap // 128, 128, DN_W),
            "bm": np.ascontiguousarray(bmv.reshape(cap // 128, 128).T),
            **common,
        })
    return nc, in_maps, gcut


def _enable_ldw_opt():
    """Re-enable the compiler's weight-load optimization (off by default in
    this container's flag set); harmless no-op if the flag isn't present."""
    try:
        from concourse.compiler_utils import get_compiler_flags, set_compiler_flags
        flags = [f.replace("--enable-ldw-opt=false", "--enable-ldw-opt=true")
                 for f in get_compiler_flags()]
        set_compiler_flags(flags)
    except Exception:
        pass


def kernel(out_gnn, batch_input, W1, b1, W2, b2):
    import concourse.bass_utils as bass_utils

    _enable_ldw_opt()
    nc, in_maps, gcut = _prepare(out_gnn, batch_input, W1, b1, W2, b2)

    trace_dir = os.environ.get("NODEATT_TRACE_DIR")
    kw = {}
    if trace_dir:
        kw = {"trace": True, "tmpdir": trace_dir}
    res = bass_utils.run_bass_kernel_spmd(
        nc, in_maps, core_ids=list(range(N_CORES)), **kw)
    if trace_dir:
        kernel.last_exec_time_ns = res.exec_time_ns
        kernel.last_results = res

    out = np.zeros((NUM_GRAPHS, D), dtype=np.float32)
    for c in range(N_CORES):
        span = int(gcut[c + 1] - gcut[c])
        if span > 0:
            out[gcut[c]:gcut[c + 1]] = res.results[c]["out"][:span]
    return out


# revision 20
# speedup vs baseline: 1.0084x; 1.0084x over previous
"""Trainium2 Bass kernel for nn_NodeAttDiff (segment-reduce node attention).

Math (reference):
    e1, e2 = out_gnn[:N], out_gnn[N:]          # N = 200000, D = 256
    diff   = e1 - e2
    h      = relu([e1 e2 diff] @ W1 + b1)      # folded: e1@WA + e2@WB, WA=W1a+W1c, WB=W1b-W1c
    raw    = (h @ W2 + b2)[:, 0]
    att    = segment_softmax(raw, batch)       # 512 contiguous segments (batch sorted)
    out    = segment_sum(att[:,None] * diff)   # [512, 256]

Device strategy (8 cores, graph-partitioned data parallel):
    - 64 graphs / core; each core gets its contiguous node slice (padded to a
      common capacity, pad nodes carry out-of-range segment id -> dropped).
    - Softmax max-subtraction is skipped (raw is O(5); exp is safe in fp32) and
      normalization is algebraic:  out_g = (sum_n w_n diff_n) / (sum_n w_n),
      w_n = exp(raw_n + b2)  -- so no per-node att materialization is needed.
    - Host ships THREE tensors per core (fp16, ~39 MB total vs 360 GB/s HBM):
        e1t/e2t feature-major (merged dram tensor e12) feed the z GEMM;
        dn node-major [cap/128,128,258] = [diff | 1 | 0] rows feeds the
        segment matmul directly (no PE transposes, no DVE sub/copies; the
        denominator "ones" column ships inside the same DMA).
    - Software pipeline, one group = 2x512-node tiles, all cross-engine
      dependencies get >= 1 full group of slack so the PE never stalls:
        iteration g:  PE:  raw(g-1) x4 -> seg(g-2) x8 -> z(g) x8
                      ACT: exp(g-1) x2, relu(g) x2 (PSUM pair -> fp16 SBUF)
                      DMA: e12(g+2), dn(g+2); XBAR-transpose ew(g-1) -> ewt
                      DVE: Sw(g-1) = (iota == seg_id) * ewt  x8
      The ew row pair [16,1024] is transposed by the DMA crossbar (14 ns/tile)
      instead of PE outer-product matmuls; PSUM = 3x z (2 banks each) +
      raw + seg = 8 banks.
    - Tail: out = seg[:,0:256] * recip(max(seg[:,256], eps)), DMA out [gw,256].
"""

import os
import numpy as np

NUM_GRAPHS = 512
N_CORES = 8
D = 256
TILE_N = 512  # nodes per tile
DN_W = D + 2  # diff row + [1, 0]


_CACHE = {}


def _build_program(cap: int, gw: int, use_b1: bool = False):
    """Build + compile the SPMD Bass program; `cap` nodes and a `gw`-graph
    window per core."""
    if (cap, gw, use_b1) in _CACHE:
        return _CACHE[(cap, gw, use_b1)]

    from contextlib import ExitStack
    import concourse.bass as bass
    import concourse.tile as tile
    import concourse.bacc as bacc
    import concourse.mybir as mybir

    f32 = mybir.dt.float32
    f16 = mybir.dt.float16
    AF = mybir.ActivationFunctionType
    ALU = mybir.AluOpType

    assert cap % (2 * TILE_N) == 0
    n_grp = cap // (2 * TILE_N)
    n_tiles = cap // TILE_N
    n_cols = cap // 128  # bm columns
    n_blk = cap // 128   # 128-node blocks

    nc = bacc.Bacc("TRN2", target_bir_lowering=False, debug=False,
                   num_devices=N_CORES)

    e12_d = nc.dram_tensor("e12", [2, 2, 128, cap], f16, kind="ExternalInput").ap()
    dn_d = nc.dram_tensor("dn", [n_blk, 128, DN_W], f16, kind="ExternalInput").ap()
    bm_d = nc.dram_tensor("bm", [128, n_cols], f32, kind="ExternalInput").ap()
    wa_d = nc.dram_tensor("wa", [2, 2, 128, 128], f16, kind="ExternalInput").ap()
    wb_d = nc.dram_tensor("wb", [2, 2, 128, 128], f16, kind="ExternalInput").ap()
    w2_d = nc.dram_tensor("w2", [2, 128, 2], f16, kind="ExternalInput").ap()
    b1_d = nc.dram_tensor("b1", [2, 128, 1], f32, kind="ExternalInput").ap()
    b2_d = nc.dram_tensor("b2", [1, 1], f32, kind="ExternalInput").ap()
    iota_d = nc.dram_tensor("iota", [128, gw], f16, kind="ExternalInput").ap()
    out_d = nc.dram_tensor("out", [gw, D], f32, kind="ExternalOutput").ap()

    with tile.TileContext(nc) as tc:
        with ExitStack() as ctx:
            consts = ctx.enter_context(tc.tile_pool(name="consts", bufs=1))
            epool = ctx.enter_context(tc.tile_pool(name="epool", bufs=6))
            dpool = ctx.enter_context(tc.tile_pool(name="dpool", bufs=7))
            hpool = ctx.enter_context(tc.tile_pool(name="hpool", bufs=6))
            spool = ctx.enter_context(tc.tile_pool(name="spool", bufs=6))
            zpool = ctx.enter_context(
                tc.tile_pool(name="zpool", bufs=3, space=bass.MemorySpace.PSUM))
            rawpool = ctx.enter_context(
                tc.tile_pool(name="rawpool", bufs=1, space=bass.MemorySpace.PSUM))
            segpool = ctx.enter_context(
                tc.tile_pool(name="segpool", bufs=1, space=bass.MemorySpace.PSUM))

            # ---- constants (wa/wb first: first z matmul needs them) ----
            wa = consts.tile([128, 2, 2, 128], f16, tag="wa")
            wb = consts.tile([128, 2, 2, 128], f16, tag="wb")
            w2 = consts.tile([128, 2, 2], f16, tag="w2")
            b1 = consts.tile([128, 2, 1], f32, tag="b1")
            b2 = consts.tile([1, 1], f32, tag="b2")
            iota = consts.tile([128, gw], f16, tag="iota")
            bm = consts.tile([128, n_cols], f32, tag="bm")
            nc.sync.dma_start(wa[:], wa_d.rearrange("k m p n -> p k m n"))
            nc.sync.dma_start(wb[:], wb_d.rearrange("k m p n -> p k m n"))

            # seg layout: cols 0:256 weighted diff sums, col 256 exp-sums
            seg = segpool.tile([gw, DN_W], f32, tag="seg")

            e_tiles = {}
            d_tiles = {}

            def issue_dma(g):
                if g >= n_grp:
                    return
                e12 = epool.tile([128, 2, 2, 2 * TILE_N], f16, tag="e12")
                dn = dpool.tile([128, 8, DN_W], f16, tag="dn")
                gsl = bass.ts(g, 2 * TILE_N)
                nc.sync.dma_start(
                    e12[:], e12_d[:, :, :, gsl].rearrange("s k p n -> p s k n"))
                nc.sync.dma_start(dn[:], dn_d[bass.ts(g, 8)].rearrange("b p f -> p b f"))
                e_tiles[g] = e12
                d_tiles[g] = dn

            issue_dma(0)
            # remaining consts after the first data group
            nc.sync.dma_start(w2[:], w2_d.rearrange("m p n -> p m n"))
            nc.sync.dma_start(b1[:], b1_d.rearrange("m p n -> p m n"))
            nc.sync.dma_start(b2[:], b2_d[:])
            nc.sync.dma_start(iota[:], iota_d[:])
            nc.sync.dma_start(bm[:], bm_d[:])
            issue_dma(1)

            h_tiles = {}
            ewt_tiles = {}
            sw_tiles = {}
            zc_tiles = {}
            seg_started = [False]

            # every cross-engine dependency gets >= 1 full iteration of slack:
            # raw(g) consumes h(g) two iterations after relu(g) was issued,
            # seg(g) consumes sw(g) one full iteration after the DVE batch
            # that produced it, so the in-order PE queue never stalls and the
            # tensor engine stays at its ramped p-state.
            for it in range(n_grp + 3):
                gz = it          # z / relu phase
                gr = it - 2      # raw / exp / xbar / sw phase
                gs = it - 3      # seg accumulate phase

                # ---- prefetch group g+2 (ahead of this iteration's XBAR so
                # the in-order sync queue keeps the DMA runahead at 2 groups)
                issue_dma(it + 2)

                # ---- raw(g-1): 4 matmuls [2,512] (h has a full group of slack)
                if 0 <= gr < n_grp:
                    rawp = rawpool.tile([64, TILE_N], f32, tag="raw")
                    for ti in range(2):
                        h = h_tiles[(gr, ti)]
                        nc.tensor.matmul(rawp[32 * ti:32 * ti + 2, :],
                                         w2[:, 0, :], h[:, 0, :],
                                         start=True, stop=False,
                                         skip_group_check=True)
                        nc.tensor.matmul(rawp[32 * ti:32 * ti + 2, :],
                                         w2[:, 1, :], h[:, 1, :],
                                         start=False, stop=True,
                                         skip_group_check=True)
                    # ew = exp(raw + b2) -> rows 0 of a [16, 1024] strip
                    ew = spool.tile([16, 2, TILE_N], f16, tag="ew")
                    for ti in range(2):
                        nc.scalar.activation(ew[0:1, ti, :],
                                             rawp[32 * ti:32 * ti + 1, :],
                                             AF.Exp, bias=b2[:], scale=1.0)
                    # ewt[p, blk, 0] = ew[0, blk*128 + p]  (DMA crossbar).
                    # Dispatched from the scalar hwdge queue: it sits right
                    # behind its producer exp there, keeping the sync queue
                    # free for data DMAs (on sync it blocked the prefetch).
                    ewt = spool.tile([128, 8, 16], f16, tag="ewt")
                    nc.scalar.dma_start_transpose(ewt[:], ew[:])
                    # fp32 copy of the 8 useful columns (tensor_scalar scalar2
                    # must be fp32 for is_equal)
                    ewt32 = spool.tile([128, 8, 1], f32, tag="ewt32")
                    nc.vector.tensor_copy(ewt32[:], ewt[:, :, 0:1])
                    ewt_tiles[gr] = ewt32

                # ---- seg(g-2): 8 matmuls into the whole-core accumulator
                if 0 <= gs < n_grp:
                    dnt = d_tiles.pop(gs)
                    for ti in range(2):
                        sw = sw_tiles.pop((gs, ti))
                        for b in range(4):
                            nc.tensor.matmul(seg[:], sw[:, b, :],
                                             dnt[:, 4 * ti + b, :],
                                             start=not seg_started[0],
                                             stop=(gs == n_grp - 1 and ti == 1
                                                   and b == 3),
                                             skip_group_check=True)
                            seg_started[0] = True

                # ---- z(g): 8 matmuls, each weight chunk streamed to both tiles
                if gz < n_grp:
                    e12 = e_tiles.pop(gz)
                    zc = [zpool.tile([128, 2, TILE_N], f32, tag="zr",
                                     name=f"z_{gz}_{ti}") for ti in range(2)]
                    for m in range(2):
                        for wi in range(4):
                            wmat = wa if wi < 2 else wb
                            s = wi // 2  # e1 or e2
                            k = wi % 2
                            for ti in range(2):
                                nc.tensor.matmul(
                                    zc[ti][:, m, :], wmat[:, k, m, :],
                                    e12[:, s, k, bass.ts(ti, TILE_N)],
                                    start=(wi == 0), stop=(wi == 3))
                    # h = relu(z + b1); fast path (b1 == 0) fuses both banks
                    for ti in range(2):
                        h = hpool.tile([128, 2, TILE_N], f16, tag="h")
                        if use_b1:
                            for m in range(2):
                                nc.scalar.activation(h[:, m, :], zc[ti][:, m, :],
                                                     AF.Relu, bias=b1[:, m, :],
                                                     scale=1.0)
                        else:
                            nc.scalar.activation(h[:], zc[ti][:], AF.Relu,
                                                 scale=1.0)
                        h_tiles[(gz, ti)] = h
                    zc_tiles[gz] = zc

                # ---- Sw(g-1) on DVE (after the XBAR above)
                if 0 <= gr < n_grp:
                    ewt = ewt_tiles.pop(gr)
                    for ti in range(2):
                        t = 2 * gr + ti
                        sw = spool.tile([128, 4, gw], f16, tag="sw")
                        for b in range(4):
                            nc.vector.tensor_scalar(
                                sw[:, b, :], iota[:],
                                bm[:, 4 * t + b:4 * t + b + 1],
                                ewt[:, 4 * ti + b, :],
                                op0=ALU.is_equal, op1=ALU.mult)
                        sw_tiles[(gr, ti)] = sw

            # tail: out = seg[:, 0:256] / max(seg[:, 256], eps)
            ssum = spool.tile([gw, 1], f32, tag="ssum")
            nc.vector.tensor_scalar_max(ssum[:], seg[:, D:D + 1], 1e-30)
            rec = spool.tile([gw, 1], f32, tag="rec")
            nc.vector.reciprocal(rec[:], ssum[:])
            ot = spool.tile([gw, D], f32, tag="ot")
            nc.vector.tensor_scalar_mul(ot[:], seg[:, 0:D], rec[:])
            nc.sync.dma_start(out_d[:], ot[:])

    nc.compile()
    _CACHE[(cap, gw, use_b1)] = nc
    return nc


def _prepare(out_gnn, batch_input, W1, b1, W2, b2):
    out_gnn = np.asarray(out_gnn, dtype=np.float32)
    batch = np.asarray(batch_input, dtype=np.int64)
    W1 = np.asarray(W1, dtype=np.float32)
    b1 = np.asarray(b1, dtype=np.float32)
    W2 = np.asarray(W2, dtype=np.float32)
    b2 = np.asarray(b2, dtype=np.float32)
    use_b1 = bool(b1.any())

    half = out_gnn.shape[0] // 2
    batch = batch[:half]
    e1_all, e2_all = out_gnn[:half], out_gnn[half:]

    # Node-balanced, graph-aligned contiguous cuts. Core c handles graphs
    # [gcut[c], gcut[c+1]) and the matching contiguous node range. The
    # sorted batch may populate only a prefix of the 512 graphs, so cuts
    # are chosen by node mass, not by fixed graph ranges.
    counts = np.bincount(batch, minlength=NUM_GRAPHS)
    ccum = np.concatenate([[0], np.cumsum(counts)])  # node offset per graph
    # only graphs up to the last populated one get device windows; trailing
    # empty graphs stay host-side zeros
    g_used = int(np.max(np.nonzero(counts)[0])) + 1 if counts.any() else 1
    gcut = np.zeros(N_CORES + 1, dtype=np.int64)
    gcut[N_CORES] = g_used
    for c in range(1, N_CORES):
        g = int(np.searchsorted(ccum, ccum[g_used] * c / N_CORES, side="left"))
        gcut[c] = min(max(g, gcut[c - 1]), g_used)
    spans = gcut[1:] - gcut[:-1]
    if spans.max() > 128:
        # node-balanced cuts gave an oversized graph window (pathological
        # distribution) -- fall back to an even graph split of [0, g_used)
        gcut = np.round(np.linspace(0, g_used, N_CORES + 1)).astype(np.int64)
        spans = gcut[1:] - gcut[:-1]
        if spans.max() > 128:
            raise ValueError(f"graph window {spans.max()} > 128 unsupported")

    nbounds = ccum[gcut]  # node boundaries per core
    gw = int(max(2, ((spans.max() + 1) // 2) * 2))
    max_n = int((nbounds[1:] - nbounds[:-1]).max())
    grp = 2 * TILE_N
    cap = max(grp, ((max_n + grp - 1) // grp) * grp)

    nc = _build_program(cap, gw, use_b1)

    # host-folded MLP weights (fp64 for exactness, then fp16)
    W1a = W1[0:D].astype(np.float64)
    W1b = W1[D:2 * D].astype(np.float64)
    W1c = W1[2 * D:3 * D].astype(np.float64)
    WA = (W1a + W1c).astype(np.float32)
    WB = (W1b - W1c).astype(np.float32)

    def chunk4(w):  # [256,256] -> [ki, mo, 128, 128]
        return np.ascontiguousarray(
            w.astype(np.float16).reshape(2, 128, 2, 128).transpose(0, 2, 1, 3))

    common = {
        "wa": chunk4(WA),
        "wb": chunk4(WB),
        "w2": np.ascontiguousarray(np.concatenate(
            [W2.astype(np.float16).reshape(2, 128, 1),
             np.zeros((2, 128, 1), np.float16)], axis=2)),
        "b1": np.ascontiguousarray(b1.reshape(2, 128, 1)),
        "b2": b2.reshape(1, 1).astype(np.float32),
        "iota": np.broadcast_to(np.arange(gw, dtype=np.float16), (128, gw)).copy(),
    }

    in_maps = []
    for c in range(N_CORES):
        s, e = int(nbounds[c]), int(nbounds[c + 1])
        n_c = e - s
        e12 = np.zeros((2, 2, 128, cap), dtype=np.float16)
        e12[0, :, :, :n_c] = e1_all[s:e].astype(np.float16).T.reshape(2, 128, n_c)
        e12[1, :, :, :n_c] = e2_all[s:e].astype(np.float16).T.reshape(2, 128, n_c)
        dn = np.zeros((cap, DN_W), dtype=np.float16)
        dn[:n_c, :D] = (e1_all[s:e] - e2_all[s:e]).astype(np.float16)
        dn[:, D] = 1.0  # denominator ones column (pad rows get Sw == 0 anyway)
        bmv = np.full(cap, 999.0, dtype=np.float32)
        bmv[:n_c] = (batch[s:e] - gcut[c]).astype(np.float32)
        in_maps.append({
            "e12": e12,
            "dn": dn.reshape(cap // 128, 128, DN_W),
            "bm": np.ascontiguousarray(bmv.reshape(cap // 128, 128).T),
            **common,
        })
    return nc, in_maps, gcut


def _enable_ldw_opt():
    """Re-enable the compiler's weight-load optimization (off by default in
    this container's flag set); harmless no-op if the flag isn't present."""
    try:
        from concourse.compiler_utils import get_compiler_flags, set_compiler_flags
        flags = [f.replace("--enable-ldw-opt=false", "--enable-ldw-opt=true")
                 for f in get_compiler_flags()]
        set_compiler_flags(flags)
    except Exception:
        pass


def kernel(out_gnn, batch_input, W1, b1, W2, b2):
    import concourse.bass_utils as bass_utils

    _enable_ldw_opt()
    nc, in_maps, gcut = _prepare(out_gnn, batch_input, W1, b1, W2, b2)

    trace_dir = os.environ.get("NODEATT_TRACE_DIR")
    kw = {}
    if trace_dir:
        kw = {"trace": True, "tmpdir": trace_dir}
    res = bass_utils.run_bass_kernel_spmd(
        nc, in_maps, core_ids=list(range(N_CORES)), **kw)
    if trace_dir:
        kernel.last_exec_time_ns = res.exec_time_ns
        kernel.last_results = res

    out = np.zeros((NUM_GRAPHS, D), dtype=np.float32)
    for c in range(N_CORES):
        span = int(gcut[c + 1] - gcut[c])
        if span > 0:
            out[gcut[c]:gcut[c + 1]] = res.results[c]["out"][:span]
    return out


# revision 27
# speedup vs baseline: 1.5558x; 1.5428x over previous
"""Trainium2 Bass kernel for nn_NodeAttDiff (segment-reduce node attention).

Math (reference):
    e1, e2 = out_gnn[:N], out_gnn[N:]          # N = 200000, D = 256
    diff   = e1 - e2
    h      = relu([e1 e2 diff] @ W1 + b1)      # folded: e1@WA + e2@WB, WA=W1a+W1c, WB=W1b-W1c
    raw    = (h @ W2 + b2)[:, 0]
    att    = segment_softmax(raw, batch)       # 512 contiguous segments (batch sorted)
    out    = segment_sum(att[:,None] * diff)   # [512, 256]

Device strategy (8 cores, graph-partitioned data parallel):
    - 64 graphs / core; each core gets its contiguous node slice (padded to a
      common capacity, pad nodes carry out-of-range segment id -> dropped).
    - Softmax max-subtraction is skipped (raw is O(5); exp is safe in fp32) and
      normalization is algebraic:  out_g = (sum_n w_n diff_n) / (sum_n w_n),
      w_n = exp(raw_n + b2)  -- so no per-node att materialization is needed.
    - Host ships THREE tensors per core (fp16, ~39 MB total vs 360 GB/s HBM):
        e1t/e2t feature-major (merged dram tensor e12) feed the z GEMM;
        dn node-major [cap/128,128,258] = [diff | 1 | 0] rows feeds the
        segment matmul directly (no PE transposes, no DVE sub/copies; the
        denominator "ones" column ships inside the same DMA).
    - Software pipeline, one group = 2x512-node tiles, all cross-engine
      dependencies get >= 1 full group of slack so the PE never stalls:
        iteration g:  PE:  raw(g-1) x4 -> seg(g-2) x8 -> z(g) x8
                      ACT: exp(g-1) x2, relu(g) x2 (PSUM pair -> fp16 SBUF)
                      DMA: e12(g+2), dn(g+2); XBAR-transpose ew(g-1) -> ewt
                      DVE: Sw(g-1) = (iota == seg_id) * ewt  x8
      The ew row pair [16,1024] is transposed by the DMA crossbar (14 ns/tile)
      instead of PE outer-product matmuls; PSUM = 3x z (2 banks each) +
      raw + seg = 8 banks.
    - Tail: out = seg[:,0:256] * recip(max(seg[:,256], eps)), DMA out [gw,256].
"""

import os
import numpy as np

NUM_GRAPHS = 512
N_CORES = 8
D = 256
TILE_N = 512  # nodes per tile
DN_W = D + 2  # diff row + [1, 0]


_CACHE = {}


def _build_program(cap: int, gw: int, use_b1: bool = False):
    """Build + compile the SPMD Bass program; `cap` nodes and a `gw`-graph
    window per core."""
    if (cap, gw, use_b1) in _CACHE:
        return _CACHE[(cap, gw, use_b1)]

    from contextlib import ExitStack
    import concourse.bass as bass
    import concourse.tile as tile
    import concourse.bacc as bacc
    import concourse.mybir as mybir

    f32 = mybir.dt.float32
    f16 = mybir.dt.float16
    AF = mybir.ActivationFunctionType
    ALU = mybir.AluOpType

    assert cap % (2 * TILE_N) == 0
    n_grp = cap // (2 * TILE_N)
    n_tiles = cap // TILE_N
    n_cols = cap // 128  # bm columns
    n_blk = cap // 128   # 128-node blocks

    nc = bacc.Bacc("TRN2", target_bir_lowering=False, debug=False,
                   num_devices=N_CORES)

    e12_d = nc.dram_tensor("e12", [2, 2, 128, cap], f16, kind="ExternalInput").ap()
    dn_d = nc.dram_tensor("dn", [n_blk, 128, DN_W], f16, kind="ExternalInput").ap()
    bm_d = nc.dram_tensor("bm", [128, n_cols], f32, kind="ExternalInput").ap()
    wa_d = nc.dram_tensor("wa", [2, 2, 128, 128], f16, kind="ExternalInput").ap()
    wb_d = nc.dram_tensor("wb", [2, 2, 128, 128], f16, kind="ExternalInput").ap()
    w2_d = nc.dram_tensor("w2", [2, 128, 2], f16, kind="ExternalInput").ap()
    b1_d = nc.dram_tensor("b1", [2, 128, 1], f32, kind="ExternalInput").ap()
    b2_d = nc.dram_tensor("b2", [1, 1], f32, kind="ExternalInput").ap()
    iota_d = nc.dram_tensor("iota", [128, gw], f16, kind="ExternalInput").ap()
    i4_d = nc.dram_tensor("i4", [1, 16], f16, kind="ExternalInput").ap()
    out_d = nc.dram_tensor("out", [gw, D], f32, kind="ExternalOutput").ap()

    with tile.TileContext(nc) as tc:
        with ExitStack() as ctx:
            consts = ctx.enter_context(tc.tile_pool(name="consts", bufs=1))
            epool = ctx.enter_context(tc.tile_pool(name="epool", bufs=6))
            dpool = ctx.enter_context(tc.tile_pool(name="dpool", bufs=7))
            hpool = ctx.enter_context(tc.tile_pool(name="hpool", bufs=6))
            spool = ctx.enter_context(tc.tile_pool(name="spool", bufs=6))
            zpool = ctx.enter_context(
                tc.tile_pool(name="zpool", bufs=2, space=bass.MemorySpace.PSUM))
            rawpool = ctx.enter_context(
                tc.tile_pool(name="rawpool", bufs=1, space=bass.MemorySpace.PSUM))
            etpool = ctx.enter_context(
                tc.tile_pool(name="etpool", bufs=1, space=bass.MemorySpace.PSUM))
            segpool = ctx.enter_context(
                tc.tile_pool(name="segpool", bufs=1, space=bass.MemorySpace.PSUM))

            # ---- constants (wa/wb first: first z matmul needs them) ----
            wa = consts.tile([128, 2, 2, 128], f16, tag="wa")
            wb = consts.tile([128, 2, 2, 128], f16, tag="wb")
            w2 = consts.tile([128, 2, 2], f16, tag="w2")
            b1 = consts.tile([128, 2, 1], f32, tag="b1")
            b2 = consts.tile([1, 1], f32, tag="b2")
            iota = consts.tile([128, gw], f16, tag="iota")
            i4 = consts.tile([1, 16], f16, tag="i4")
            bm = consts.tile([128, n_cols], f32, tag="bm")
            nc.sync.dma_start(wa[:], wa_d.rearrange("k m p n -> p k m n"))
            nc.sync.dma_start(wb[:], wb_d.rearrange("k m p n -> p k m n"))

            # seg layout: cols 0:256 weighted diff sums, col 256 exp-sums
            seg = segpool.tile([gw, DN_W], f32, tag="seg")

            e_tiles = {}
            d_tiles = {}

            def issue_dma(g):
                if g >= n_grp:
                    return
                e12 = epool.tile([128, 2, 2, 2 * TILE_N], f16, tag="e12")
                dn = dpool.tile([128, 8, DN_W], f16, tag="dn")
                gsl = bass.ts(g, 2 * TILE_N)
                nc.sync.dma_start(
                    e12[:], e12_d[:, :, :, gsl].rearrange("s k p n -> p s k n"))
                nc.sync.dma_start(dn[:], dn_d[bass.ts(g, 8)].rearrange("b p f -> p b f"))
                e_tiles[g] = e12
                d_tiles[g] = dn

            issue_dma(0)
            # remaining consts after the first data group
            nc.sync.dma_start(w2[:], w2_d.rearrange("m p n -> p m n"))
            nc.sync.dma_start(b1[:], b1_d.rearrange("m p n -> p m n"))
            nc.sync.dma_start(b2[:], b2_d[:])
            nc.sync.dma_start(iota[:], iota_d[:])
            nc.sync.dma_start(i4[:], i4_d[:])
            nc.sync.dma_start(bm[:], bm_d[:])
            issue_dma(1)

            h_tiles = {}
            ewt_tiles = {}
            sw_tiles = {}
            zc_tiles = {}
            seg_started = [False]

            # every cross-engine dependency gets >= 1 full iteration of slack:
            # raw(g) consumes h(g) two iterations after relu(g) was issued,
            # seg(g) consumes sw(g) one full iteration after the DVE batch
            # that produced it, so the in-order PE queue never stalls and the
            # tensor engine stays at its ramped p-state.
            for it in range(n_grp + 3):
                gz = it          # z / relu phase
                gr = it - 2      # raw / exp / xbar / sw phase
                gs = it - 3      # seg accumulate phase

                # ---- prefetch group g+2 (ahead of this iteration's XBAR so
                # the in-order sync queue keeps the DMA runahead at 2 groups)
                issue_dma(it + 2)

                # ---- raw(g-2): 4 matmuls [2,512] (h has 2 groups of slack)
                if 0 <= gr < n_grp:
                    rawp = rawpool.tile([64, TILE_N], f32, tag="raw")
                    ewt_ps = etpool.tile([128, 8], f32, tag="ewt_ps")
                    ew = []
                    for ti in range(2):
                        h = h_tiles.pop((gr, ti))
                        nc.tensor.matmul(rawp[32 * ti:32 * ti + 2, :],
                                         w2[:, 0, :], h[:, 0, :],
                                         start=True, stop=False,
                                         skip_group_check=True)
                        nc.tensor.matmul(rawp[32 * ti:32 * ti + 2, :],
                                         w2[:, 1, :], h[:, 1, :],
                                         start=False, stop=True,
                                         skip_group_check=True)
                        # ew = exp(raw + b2) -> SBUF row [1, 512]
                        ewr = spool.tile([1, TILE_N], f16, tag="ew")
                        nc.scalar.activation(ewr[:], rawp[32 * ti:32 * ti + 1, :],
                                             AF.Exp, bias=b2[:], scale=1.0)
                        ew.append(ewr)
                    # ewT [128, 4] per tile: outer products with I4 rows
                    for ti in range(2):
                        for b in range(4):
                            nc.tensor.matmul(ewt_ps[:, 4 * ti:4 * ti + 4],
                                             ew[ti][:, bass.ts(b, 128)],
                                             i4[:, bass.ts(b, 4)],
                                             start=(ti == 0 and b == 0),
                                             stop=(ti == 1 and b == 3),
                                             skip_group_check=True)
                    ewt32 = spool.tile([128, 8], f32, tag="ewt32")
                    nc.vector.tensor_copy(ewt32[:], ewt_ps[:])
                    ewt_tiles[gr] = ewt32

                # ---- seg(g-2): 8 matmuls into the whole-core accumulator
                if 0 <= gs < n_grp:
                    dnt = d_tiles.pop(gs)
                    for ti in range(2):
                        sw = sw_tiles.pop((gs, ti))
                        for b in range(4):
                            nc.tensor.matmul(seg[:], sw[:, b, :],
                                             dnt[:, 4 * ti + b, :],
                                             start=not seg_started[0],
                                             stop=(gs == n_grp - 1 and ti == 1
                                                   and b == 3),
                                             skip_group_check=True)
                            seg_started[0] = True

                # ---- z(g): 8 matmuls, each weight chunk streamed to both tiles
                if gz < n_grp:
                    e12 = e_tiles.pop(gz)
                    zc = [zpool.tile([128, 2, TILE_N], f32, tag="zr",
                                     name=f"z_{gz}_{ti}") for ti in range(2)]
                    for m in range(2):
                        for wi in range(4):
                            wmat = wa if wi < 2 else wb
                            s = wi // 2  # e1 or e2
                            k = wi % 2
                            for ti in range(2):
                                nc.tensor.matmul(
                                    zc[ti][:, m, :], wmat[:, k, m, :],
                                    e12[:, s, k, bass.ts(ti, TILE_N)],
                                    start=(wi == 0), stop=(wi == 3))
                    # h = relu(z + b1); fast path (b1 == 0) fuses both banks
                    for ti in range(2):
                        h = hpool.tile([128, 2, TILE_N], f16, tag="h")
                        if use_b1:
                            for m in range(2):
                                nc.scalar.activation(h[:, m, :], zc[ti][:, m, :],
                                                     AF.Relu, bias=b1[:, m, :],
                                                     scale=1.0)
                        else:
                            nc.scalar.activation(h[:], zc[ti][:], AF.Relu,
                                                 scale=1.0)
                        h_tiles[(gz, ti)] = h
                    zc_tiles[gz] = zc

                # ---- Sw(g-1) on DVE (after the XBAR above)
                if 0 <= gr < n_grp:
                    ewt = ewt_tiles.pop(gr)
                    for ti in range(2):
                        t = 2 * gr + ti
                        sw = spool.tile([128, 4, gw], f16, tag="sw")
                        for b in range(4):
                            nc.vector.tensor_scalar(
                                sw[:, b, :], iota[:],
                                bm[:, 4 * t + b:4 * t + b + 1],
                                ewt[:, 4 * ti + b:4 * ti + b + 1],
                                op0=ALU.is_equal, op1=ALU.mult)
                        sw_tiles[(gr, ti)] = sw

            # tail: out = seg[:, 0:256] / max(seg[:, 256], eps)
            ssum = spool.tile([gw, 1], f32, tag="ssum")
            nc.vector.tensor_scalar_max(ssum[:], seg[:, D:D + 1], 1e-30)
            rec = spool.tile([gw, 1], f32, tag="rec")
            nc.vector.reciprocal(rec[:], ssum[:])
            ot = spool.tile([gw, D], f32, tag="ot")
            nc.vector.tensor_scalar_mul(ot[:], seg[:, 0:D], rec[:])
            nc.sync.dma_start(out_d[:], ot[:])

    nc.compile()
    _CACHE[(cap, gw, use_b1)] = nc
    return nc


def _prepare(out_gnn, batch_input, W1, b1, W2, b2):
    out_gnn = np.asarray(out_gnn, dtype=np.float32)
    batch = np.asarray(batch_input, dtype=np.int64)
    W1 = np.asarray(W1, dtype=np.float32)
    b1 = np.asarray(b1, dtype=np.float32)
    W2 = np.asarray(W2, dtype=np.float32)
    b2 = np.asarray(b2, dtype=np.float32)
    use_b1 = bool(b1.any())

    half = out_gnn.shape[0] // 2
    batch = batch[:half]
    e1_all, e2_all = out_gnn[:half], out_gnn[half:]

    # Node-balanced, graph-aligned contiguous cuts. Core c handles graphs
    # [gcut[c], gcut[c+1]) and the matching contiguous node range. The
    # sorted batch may populate only a prefix of the 512 graphs, so cuts
    # are chosen by node mass, not by fixed graph ranges.
    counts = np.bincount(batch, minlength=NUM_GRAPHS)
    ccum = np.concatenate([[0], np.cumsum(counts)])  # node offset per graph
    # only graphs up to the last populated one get device windows; trailing
    # empty graphs stay host-side zeros
    g_used = int(np.max(np.nonzero(counts)[0])) + 1 if counts.any() else 1
    gcut = np.zeros(N_CORES + 1, dtype=np.int64)
    gcut[N_CORES] = g_used
    for c in range(1, N_CORES):
        g = int(np.searchsorted(ccum, ccum[g_used] * c / N_CORES, side="left"))
        gcut[c] = min(max(g, gcut[c - 1]), g_used)
    spans = gcut[1:] - gcut[:-1]
    if spans.max() > 128:
        # node-balanced cuts gave an oversized graph window (pathological
        # distribution) -- fall back to an even graph split of [0, g_used)
        gcut = np.round(np.linspace(0, g_used, N_CORES + 1)).astype(np.int64)
        spans = gcut[1:] - gcut[:-1]
        if spans.max() > 128:
            raise ValueError(f"graph window {spans.max()} > 128 unsupported")

    nbounds = ccum[gcut]  # node boundaries per core
    gw = int(max(2, ((spans.max() + 1) // 2) * 2))
    max_n = int((nbounds[1:] - nbounds[:-1]).max())
    grp = 2 * TILE_N
    cap = max(grp, ((max_n + grp - 1) // grp) * grp)

    nc = _build_program(cap, gw, use_b1)

    # host-folded MLP weights (fp64 for exactness, then fp16)
    W1a = W1[0:D].astype(np.float64)
    W1b = W1[D:2 * D].astype(np.float64)
    W1c = W1[2 * D:3 * D].astype(np.float64)
    WA = (W1a + W1c).astype(np.float32)
    WB = (W1b - W1c).astype(np.float32)

    def chunk4(w):  # [256,256] -> [ki, mo, 128, 128]
        return np.ascontiguousarray(
            w.astype(np.float16).reshape(2, 128, 2, 128).transpose(0, 2, 1, 3))

    common = {
        "wa": chunk4(WA),
        "wb": chunk4(WB),
        "w2": np.ascontiguousarray(np.concatenate(
            [W2.astype(np.float16).reshape(2, 128, 1),
             np.zeros((2, 128, 1), np.float16)], axis=2)),
        "b1": np.ascontiguousarray(b1.reshape(2, 128, 1)),
        "b2": b2.reshape(1, 1).astype(np.float32),
        "iota": np.broadcast_to(np.arange(gw, dtype=np.float16), (128, gw)).copy(),
        "i4": np.eye(4, dtype=np.float16).reshape(1, 16),
    }

    in_maps = []
    for c in range(N_CORES):
        s, e = int(nbounds[c]), int(nbounds[c + 1])
        n_c = e - s
        e12 = np.zeros((2, 2, 128, cap), dtype=np.float16)
        e12[0, :, :, :n_c] = e1_all[s:e].astype(np.float16).T.reshape(2, 128, n_c)
        e12[1, :, :, :n_c] = e2_all[s:e].astype(np.float16).T.reshape(2, 128, n_c)
        dn = np.zeros((cap, DN_W), dtype=np.float16)
        dn[:n_c, :D] = (e1_all[s:e] - e2_all[s:e]).astype(np.float16)
        dn[:, D] = 1.0  # denominator ones column (pad rows get Sw == 0 anyway)
        bmv = np.full(cap, 999.0, dtype=np.float32)
        bmv[:n_c] = (batch[s:e] - gcut[c]).astype(np.float32)
        in_maps.append({
            "e12": e12,
            "dn": dn.reshape(cap // 128, 128, DN_W),
            "bm": np.ascontiguousarray(bmv.reshape(cap // 128, 128).T),
            **common,
        })
    return nc, in_maps, gcut


def _enable_ldw_opt():
    """Re-enable the compiler's weight-load optimization (off by default in
    this container's flag set); harmless no-op if the flag isn't present."""
    try:
        from concourse.compiler_utils import get_compiler_flags, set_compiler_flags
        flags = [f.replace("--enable-ldw-opt=false", "--enable-ldw-opt=true")
                 for f in get_compiler_flags()]
        set_compiler_flags(flags)
    except Exception:
        pass


def kernel(out_gnn, batch_input, W1, b1, W2, b2):
    import concourse.bass_utils as bass_utils

    _enable_ldw_opt()
    nc, in_maps, gcut = _prepare(out_gnn, batch_input, W1, b1, W2, b2)

    trace_dir = os.environ.get("NODEATT_TRACE_DIR")
    kw = {}
    if trace_dir:
        kw = {"trace": True, "tmpdir": trace_dir}
    res = bass_utils.run_bass_kernel_spmd(
        nc, in_maps, core_ids=list(range(N_CORES)), **kw)
    if trace_dir:
        kernel.last_exec_time_ns = res.exec_time_ns
        kernel.last_results = res

    out = np.zeros((NUM_GRAPHS, D), dtype=np.float32)
    for c in range(N_CORES):
        span = int(gcut[c + 1] - gcut[c])
        if span > 0:
            out[gcut[c]:gcut[c + 1]] = res.results[c]["out"][:span]
    return out
